# revision 50
# baseline (speedup 1.0000x reference)
"""BiLSTM (2-layer, H=512) Trainium2 Bass kernel.

Contract: kernel(**inputs) takes the FULL unsharded inputs from
setup_inputs() and returns the FULL [32, 512, 1024] float32 output.

Strategy (8 NeuronCores):
  - cores 0-3: forward direction, batch slices 0..3 (8 seqs each)
  - cores 4-7: backward direction, batch slices 0..3
  - two SPMD launches (layer 0, layer 1); host reshuffles between them.
  - backward cores run the IDENTICAL program on time-reversed inputs
    (host flips t), so one Bass program per layer serves all 8 cores.

Math layout per core (B_local=8, T=512, H=512, G=4H=2048):
  - recurrent matmul computed transposed: z^T[G, B] = Wh^T @ h^T via 64
    weights-stationary matmuls [K=128] x [M=128] x [N=8]; gate dim lands
    on partitions so all elementwise gate math runs 128-wide.
  - the per-step cost is LDWEIGHTS-bound (stationary loads at ~P cols /
    (elems-per-cycle) / 1.2GHz). Wh is stored float8e3 (E3M4) so FWL loads
    4 elems/cycle instead of bf16's 2 - this halves the recurrence floor.
    Wh/Wi/b are pre-scaled by WH_SCALE=32 to center Wh in the e3m4 normal
    range; 1/32 is folded into the gate activations' input scale (the c/h
    math stays in true scale). Measured end-to-end rel_l2 ~9.3e-3.
  - gates are column-reordered (g, i, f, o) so sigmoid is one [128,64] op
    and only o's add+sigmoid+mul sit on the per-step critical path.
  - input projection xw^T = 32*(Wi^T @ x^T + b) is computed into an SBUF
    fp32 ring buffer, interleaved with the recurrence (bf16 operands).
  - state c fp32; h stored bf16 for the next-step matmul operand.
  - kernel_merged() is an alternative single-launch path (layer0 ->
    on-device pairwise AllGather + time-reversal -> layer1); correct but
    measured neutral-to-slower than two launches, so not the default.
"""

import os
import sys
from contextlib import ExitStack, nullcontext

import numpy as np

sys.path.insert(0, "/opt/trn_rl_repo")

import ml_dtypes  # noqa: E402

import concourse.bass as bass  # noqa: E402
import concourse.tile as tile  # noqa: E402
from concourse import bacc, mybir  # noqa: E402
from concourse import bass_utils  # noqa: E402
from concourse import tile_rust  # noqa: E402

BF16 = mybir.dt.bfloat16
F8E3 = mybir.dt.float8e3
F32 = mybir.dt.float32
NP_BF16 = ml_dtypes.bfloat16
NP_E3M4 = ml_dtypes.float8_e3m4
AF = mybir.ActivationFunctionType

# Wh is stored as float8e3 (E3M4) scaled by WH_SCALE so its ~N(0, 0.05^2)
# values land in the e3m4 normal range; Wi/b carry the same scale so the
# PSUM + ring add stays consistent, and 1/WH_SCALE is folded into the gate
# activations' input scale. fp8 stationary operands halve LDWEIGHTS time
# (FWL reads 32 bits/cycle), which is the recurrence bottleneck.
WH_SCALE = 32.0

B_GLOBAL = 32
T_FULL = 512
D0 = 256
H = 512
G = 4 * H          # 2048
BL = 8             # batch per core
N_MC = 16          # gate-dim chunks of 128
N_KC = 4           # hidden-dim chunks of 128
BLK = 32           # proj block: steps of xw produced per block (layer 0)
BLK1 = int(os.environ.get("BLSTM_BLK1", "32"))  # layer-1 proj block
RING = 128         # xw ring depth (steps)
HRING = 128        # layer-1 h history ring depth (steps)
HCHUNK = 64        # layer-1 h history DMA-out chunk (steps)
WARM_BLOCKS = 2    # proj blocks emitted before the recurrence starts: the
                   # warm region executes ahead of step 0, so it is pure
                   # startup latency; 2 blocks = 64 steps of ring slack,
                   # double the drip's 1-block production granularity

# gate reorder: reference order (i, f, g, o) -> kernel order (g, i, f, o):
# mc 0-3 = g, mc 4-7 = i, mc 8-11 = f, mc 12-15 = o. The o-gate matmuls run
# last so only its add+sigmoid+mul sit on the per-step critical path.
_PERM = np.concatenate([np.arange(1024, 1536), np.arange(0, 1024),
                        np.arange(1536, 2048)])

_PROGRAM_CACHE = {}

# test hooks: per-launch BassKernelResults (trace mode) / wall seconds
LAST_RESULTS = []
LAST_WALL = []
TRACE = bool(int(os.environ.get("BLSTM_TRACE", "0")))

# o-gate tail split: pipeline the o-gate per hidden-chunk (kc) so the next
# step's matmuls can start on h slices as they complete, instead of waiting
# for the full [128, 32] o-tail. Needs 4 single-buffer psO banks.
SPLIT_O = bool(int(os.environ.get("BLSTM_SPLIT_O", "0")))

# schedule recurrence ops at priority 0 so projection work only fills real
# gaps: the Tile scheduler's cost model doesn't model LDWEIGHTS, so by
# default it may interleave projection matmuls into the recurrence chain.
HIPRI = bool(int(os.environ.get("BLSTM_HIPRI", "0")))


def _emit_layer(tc, aps, dc_n, T, layer, xsrc=None, xsrc_deps=None,
                dma_sink=None):
    nc = tc.nc
    xT, wh, wi, bT, hout = aps
    if xsrc is None:
        def xsrc(j, dc, blk):
            return xT[dc, :, j * blk * 8:(j + 1) * blk * 8]
    ring_depth = min(RING, T)
    hring = min(HRING, T)
    blk = min(BLK1 if layer == 1 else BLK, T)
    n_blk = (T + blk - 1) // blk

    ctx = ExitStack()
    const = ctx.enter_context(tc.tile_pool(name="const", bufs=1))
    xin = ctx.enter_context(tc.tile_pool(name="xin", bufs=2 * dc_n))
    pps = ctx.enter_context(tc.tile_pool(name="pps", bufs=2, space="PSUM"))
    gb = 1 if SPLIT_O else 2
    rpsG = ctx.enter_context(tc.tile_pool(name="rpsG", bufs=gb, space="PSUM"))
    rpsIF = ctx.enter_context(tc.tile_pool(name="rpsIF", bufs=gb, space="PSUM"))
    rpsO = ctx.enter_context(tc.tile_pool(name="rpsO", bufs=gb, space="PSUM"))
    ztmp = ctx.enter_context(tc.tile_pool(name="ztmp", bufs=3))
    hst = ctx.enter_context(tc.tile_pool(name="hst", bufs=3))

    with ctx:
        # ---- persistent SBUF tensors ----
        wi_sb = []
        for dc in range(dc_n):
            wt = const.tile([128, G], BF16, tag=f"wi{dc}", name=f"wi{dc}")
            nc.sync.dma_start(wt[:], wi[dc])
            wi_sb.append(wt)
        bT_sb = const.tile([128, N_MC], F32, tag="bT", name="bT_sb")
        nc.sync.dma_start(bT_sb[:], bT[:])
        h0 = const.tile([128, 32], BF16, tag="h0", name="h0_sb")
        nc.vector.memset(h0[:], 0.0)
        cT = const.tile([128, 32], F32, tag="cT", name="cT_sb")
        nc.vector.memset(cT[:], 0.0)
        ring = const.tile([128, ring_depth * 128], F32, tag="ring", name="ring_sb")
        hist_dt = BF16 if layer == 0 else F32
        hist = const.tile([128, hring * 32], hist_dt, tag="hist", name="hist_sb")
        wh_sb = []
        for kc in range(N_KC):
            wt = const.tile([128, G], F8E3, tag=f"wh{kc}", name=f"wh{kc}")
            nc.sync.dma_start(wt[:], wh[kc])
            wh_sb.append(wt)

        ringv = ring.rearrange("p (s c) -> p s c", c=128)

        # ---- projection work generator ----
        # One yield per sub-quantum of <=4 matmuls so the recurrence loop can
        # drip projection work into every step's PE idle tail.
        def proj_gen():
            for j in range(n_blk):
                xts = []
                for dc in range(dc_n):
                    xt = xin.tile([128, blk * 8], BF16, tag="xt",
                                  name=f"xt_{j}_{dc}")
                    src = xsrc(j, dc, blk)
                    dst = (xt[:] if src.ndim == 2
                           else xt[:].rearrange("p (t b) -> p t b", b=8))
                    dmai = nc.sync.dma_start(dst, src)
                    if xsrc_deps:
                        for dep in xsrc_deps:
                            tile_rust.add_dep_helper(
                                dmai.ins, dep.ins,
                                reason="proj reads exchanged h0")
                    xts.append(xt)
                s0 = (j * blk) % ring_depth
                for mc in range(N_MC):
                    ps = pps.tile([128, blk * 8], F32, tag="pps",
                                  name=f"pps_{j}_{mc}")
                    for dc in range(dc_n):
                        nc.tensor.matmul(
                            ps[:], wi_sb[dc][:, mc * 128:(mc + 1) * 128],
                            xts[dc][:],
                            start=(dc == 0), stop=(dc == dc_n - 1))
                        if dc % 4 == 3 and dc != dc_n - 1:
                            yield
                    psv = ps.rearrange("p (t b) -> p t b", b=8)
                    outv = ringv[:, s0:s0 + blk, mc * 8:(mc + 1) * 8]
                    nc.vector.tensor_scalar_add(outv, psv, bT_sb[:, mc:mc + 1])
                    yield

        gen = proj_gen()
        ypb = N_MC * (2 if dc_n > 4 else 1)   # generator yields per block
        # cap warm lookahead so it never laps the xw ring
        wb = min(WARM_BLOCKS, max(1, ring_depth // blk))
        warm = min(wb * ypb, n_blk * ypb)
        for _ in range(warm):
            next(gen, None)
        adv_acc = 0

        prev_state = None  # layer-1 bf16 state tile of previous step

        def rhs(kc, t):
            if t == 0:
                return h0[:, kc * 8:(kc + 1) * 8]
            if SPLIT_O:
                return prev_state[kc][:]
            return prev_state[:, kc * 8:(kc + 1) * 8]

        n_kc = int(os.environ.get("BLSTM_TIMING_NKC", str(N_KC)))

        def emit_mms(ps, mc0, mc1, t):
            for i, mc in enumerate(range(mc0, mc1)):
                for kc in range(n_kc):
                    nc.tensor.matmul(
                        ps[:, i * 8:(i + 1) * 8],
                        wh_sb[kc][:, mc * 128:(mc + 1) * 128],
                        rhs(kc, t),
                        start=(kc == 0), stop=(kc == n_kc - 1))

        def _emit_step(t):
            nonlocal prev_state
            st = t % ring_depth
            # gate g first (tanh overlaps i/f matmuls), o last (short tail)
            psG = rpsG.tile([128, 32], F32, tag="psG", name=f"psG_{t}")
            emit_mms(psG, 0, 4, t)
            zg = ztmp.tile([128, 32], F32, tag="zg", name=f"zg_{t}")
            nc.vector.tensor_add(zg[:], psG[:], ring[:, st * 128:st * 128 + 32])
            zgt = ztmp.tile([128, 32], F32, tag="zgt", name=f"zgt_{t}")
            nc.scalar.activation(zgt[:], zg[:], AF.Tanh, scale=1.0 / WH_SCALE)

            psIF = rpsIF.tile([128, 64], F32, tag="psIF", name=f"psIF_{t}")
            emit_mms(psIF, 4, 12, t)
            zif = ztmp.tile([128, 64], F32, tag="zif", name=f"zif_{t}")
            nc.vector.tensor_add(zif[:], psIF[:],
                                 ring[:, st * 128 + 32:st * 128 + 96])
            za = ztmp.tile([128, 64], F32, tag="za", name=f"za_{t}")
            nc.scalar.activation(za[:], zif[:], AF.Sigmoid,
                                 scale=1.0 / WH_SCALE)

            ig = ztmp.tile([128, 32], F32, tag="ig", name=f"ig_{t}")
            nc.vector.tensor_mul(ig[:], za[:, 0:32], zgt[:])
            fc = ztmp.tile([128, 32], F32, tag="fc", name=f"fc_{t}")
            nc.vector.tensor_mul(fc[:], za[:, 32:64], cT[:])
            nc.vector.tensor_add(cT[:], fc[:], ig[:])
            tct = ztmp.tile([128, 32], F32, tag="tct", name=f"tct_{t}")
            nc.scalar.activation(tct[:], cT[:], AF.Tanh)

            if SPLIT_O:
                # per-kc o-tail: each hidden-chunk's h slice completes as its
                # o-gate chain finishes, unblocking the next step's matmuls
                # for that kc while the remaining o chains still run.
                stts = []
                for q in range(4):
                    mc = 12 + q
                    psq = rpsO.tile([128, 8], F32, tag=f"psO{q}",
                                    name=f"psO_{t}_{q}")
                    for kc in range(n_kc):
                        nc.tensor.matmul(
                            psq[:], wh_sb[kc][:, mc * 128:(mc + 1) * 128],
                            rhs(kc, t),
                            start=(kc == 0), stop=(kc == n_kc - 1))
                    zoq = ztmp.tile([128, 8], F32, tag=f"zo{q}",
                                    name=f"zo_{t}_{q}")
                    nc.vector.tensor_add(
                        zoq[:], psq[:],
                        ring[:, st * 128 + 96 + q * 8:st * 128 + 96 + q * 8 + 8])
                    zsq = ztmp.tile([128, 8], F32, tag=f"zos{q}",
                                    name=f"zos_{t}_{q}")
                    nc.scalar.activation(zsq[:], zoq[:], AF.Sigmoid,
                                         scale=1.0 / WH_SCALE)
                    stq = hst.tile([128, 8], BF16, tag=f"hstate{q}",
                                   name=f"hstt_{t}_{q}")
                    nc.vector.tensor_mul(stq[:], zsq[:],
                                         tct[:, q * 8:(q + 1) * 8])
                    stts.append(stq)
                    hsq = hist[:, (t % hring) * 32 + q * 8:
                               (t % hring) * 32 + q * 8 + 8]
                    nc.vector.tensor_mul(hsq, zsq[:], tct[:, q * 8:(q + 1) * 8])
                prev_state = stts
            else:
                psO = rpsO.tile([128, 32], F32, tag="psO", name=f"psO_{t}")
                emit_mms(psO, 12, 16, t)
                zo = ztmp.tile([128, 32], F32, tag="zo", name=f"zo_{t}")
                nc.vector.tensor_add(zo[:], psO[:],
                                     ring[:, st * 128 + 96:st * 128 + 128])
                zos = ztmp.tile([128, 32], F32, tag="zos", name=f"zos_{t}")
                nc.scalar.activation(zos[:], zo[:], AF.Sigmoid,
                                     scale=1.0 / WH_SCALE)

                # bf16 state first (gates next step's matmuls), output-staging
                # write second (independent, off the critical path)
                stt = hst.tile([128, 32], BF16, tag="hstate", name=f"hstt_{t}")
                nc.vector.tensor_mul(stt[:], zos[:], tct[:])
                prev_state = stt
                hs = hist[:, (t % hring) * 32:((t % hring) + 1) * 32]
                nc.vector.tensor_mul(hs, zos[:], tct[:])
            if (t + 1) % HCHUNK == 0:
                t0 = t + 1 - HCHUNK
                c0 = (t0 % hring) * 32
                hdma = nc.sync.dma_start(hout[:, t0 * 32:(t + 1) * 32],
                                         hist[:, c0:c0 + HCHUNK * 32])
                if dma_sink is not None:
                    dma_sink.append(hdma)

        for t in range(T):
            hp = tc.high_priority() if HIPRI else nullcontext()
            with hp:
                _emit_step(t)

            # steady-state projection: drip sub-quanta into each step's tail
            adv_acc += ypb
            while adv_acc >= blk:
                next(gen, None)
                adv_acc -= blk

        # drain any remaining projection work (shouldn't happen for T=512)
        for _ in gen:
            pass

        if T % HCHUNK != 0:
            t0 = T - (T % HCHUNK)
            c0 = (t0 % hring) * 32
            hdma = nc.sync.dma_start(hout[:, t0 * 32:T * 32],
                                     hist[:, c0:c0 + (T - t0) * 32])
            if dma_sink is not None:
                dma_sink.append(hdma)


def build_layer_program(layer, T=T_FULL, reps=1):
    dc_n = 2 if layer == 0 else 8
    nc = bacc.Bacc("TRN2", target_bir_lowering=False, debug=False,
                   num_devices=8)
    xT = nc.dram_tensor("xT", [dc_n, 128, T * 8], BF16,
                        kind="ExternalInput").ap()
    wh = nc.dram_tensor("wh", [N_KC, 128, G], F8E3, kind="ExternalInput").ap()
    wi = nc.dram_tensor("wi", [dc_n, 128, G], BF16, kind="ExternalInput").ap()
    bT = nc.dram_tensor("bT", [128, N_MC], F32, kind="ExternalInput").ap()
    out_dt = BF16 if layer == 0 else F32
    hout = nc.dram_tensor("hout", [128, T * 32], out_dt,
                          kind="ExternalOutput").ap()
    with tile.TileContext(nc) as tc:
        for _ in range(reps):
            _emit_layer(tc, (xT, wh, wi, bT, hout), dc_n, T, layer)
    nc.compile()
    return nc


def _get_program(layer, T=T_FULL, reps=1):
    key = (layer, T, reps, SPLIT_O, BLK1, HIPRI)
    if key not in _PROGRAM_CACHE:
        _PROGRAM_CACHE[key] = build_layer_program(layer, T, reps)
    return _PROGRAM_CACHE[key]


def build_merged_program(T=T_FULL, dbg=False):
    """Single-launch program: layer0 -> pairwise AllGather of h0 ->
    direction-dependent reversal copy -> layer1. Cores 0-3 forward, 4-7
    backward (pair {s, 4+s} shares batch slice s)."""
    nc = bacc.Bacc("TRN2", target_bir_lowering=False, debug=False,
                   num_devices=8)
    xT = nc.dram_tensor("xT", [2, 128, T * 8], BF16,
                        kind="ExternalInput").ap()
    wh0 = nc.dram_tensor("wh0", [N_KC, 128, G], F8E3,
                         kind="ExternalInput").ap()
    wi0 = nc.dram_tensor("wi0", [2, 128, G], BF16, kind="ExternalInput").ap()
    bT0 = nc.dram_tensor("bT0", [128, N_MC], F32, kind="ExternalInput").ap()
    wh1 = nc.dram_tensor("wh1", [N_KC, 128, G], F8E3,
                         kind="ExternalInput").ap()
    wi1 = nc.dram_tensor("wi1", [8, 128, G], BF16, kind="ExternalInput").ap()
    bT1 = nc.dram_tensor("bT1", [128, N_MC], F32, kind="ExternalInput").ap()
    hout = nc.dram_tensor("hout", [128, T * 32], F32,
                          kind="ExternalOutput").ap()
    if dbg:
        dbg_all = nc.dram_tensor("dbg_all", [2, 128, T * 32], BF16,
                                 kind="ExternalOutput").ap()
        dbg_f = nc.dram_tensor("dbg_f", [2, 128, T * 32], BF16,
                               kind="ExternalOutput").ap()
        dbg_loc = nc.dram_tensor("dbg_loc", [128, T * 32], BF16,
                                 kind="ExternalOutput").ap()
    with tile.TileContext(nc) as tc:
        with ExitStack() as mctx:
            dpool = mctx.enter_context(
                tc.tile_pool(name="dramst", bufs=1, space="DRAM"))
            h0loc = dpool.tile([128, T * 32], BF16, tag="h0loc",
                               name="h0loc")
            h0all = dpool.tile([2, 128, T * 32], BF16, tag="h0all",
                               name="h0all")
            fbuf = dpool.tile([2, 128, T * 32], BF16, tag="fbuf", name="fbuf")

            # phase A: layer 0 writes h0 (core-local time) to h0loc
            staging = []
            _emit_layer(tc, (xT, wh0, wi0, bT0, h0loc[:]), 2, T, 0,
                        dma_sink=staging)

            # phase B: exchange h0 within direction pairs
            cc = tc.nc.gpsimd.collective_compute(
                "AllGather",
                mybir.AluOpType.bypass,
                replica_groups=[[0, 4], [1, 5], [2, 6], [3, 7]],
                ins=[h0loc[:].opt()],
                outs=[h0all[:].opt()],
            )
            for dma in staging:
                tile_rust.add_dep_helper(cc.ins, dma.ins,
                                         reason="gather after h0 staging")

            # fbuf[sigma] = slot sigma of h0all in THIS core's step order:
            # own-direction slot stays as-is, the other direction's slot is
            # time-reversed (its producer wrote it in opposite time order).
            cps = []

            def cp(sigma, flip):
                src = h0all[:][sigma].rearrange("p (t x) -> p t x", x=32)
                if flip:
                    src = src[:, ::-1, :]
                dst = fbuf[:][sigma].rearrange("p (t x) -> p t x", x=32)
                ck = 64
                for i in range(T // ck):
                    d = nc.sync.dma_start(dst[:, i * ck:(i + 1) * ck, :],
                                          src[:, i * ck:(i + 1) * ck, :])
                    tile_rust.add_dep_helper(d.ins, cc.ins,
                                             reason="reorder after gather")
                    cps.append(d)

            pid = nc.sync.partition_id()
            with tc.If(pid < 4) as cmp:
                cp(0, False)
                cp(1, True)
            with cmp.Else():
                cp(0, True)
                cp(1, False)

            if dbg:
                for sg in range(2):
                    for i in range(T // 64):
                        c0, c1 = i * 64 * 32, (i + 1) * 64 * 32
                        nc.sync.dma_start(dbg_all[sg, :, c0:c1],
                                          h0all[:][sg][:, c0:c1])
                        nc.sync.dma_start(dbg_f[sg, :, c0:c1],
                                          fbuf[:][sg][:, c0:c1])
                for i in range(T // 64):
                    c0, c1 = i * 64 * 32, (i + 1) * 64 * 32
                    nc.sync.dma_start(dbg_loc[:, c0:c1], h0loc[:][:, c0:c1])

            # phase C: layer 1, projection reads fbuf
            def xsrc1(j, dc, blk):
                v = fbuf[:][dc // 4].rearrange("p (t k b) -> p t k b",
                                               k=4, b=8)
                return v[:, j * blk:(j + 1) * blk, dc % 4, :]

            _emit_layer(tc, (None, wh1, wi1, bT1, hout), 8, T, 1, xsrc=xsrc1,
                        xsrc_deps=cps)
    nc.compile()
    return nc


def _get_merged(T=T_FULL):
    key = ("merged", T)
    if key not in _PROGRAM_CACHE:
        _PROGRAM_CACHE[key] = build_merged_program(T)
    return _PROGRAM_CACHE[key]


def kernel_merged(x, Wi_f0, Wh_f0, b_f0, Wi_b0, Wh_b0, b_b0,
                  Wi_f1, Wh_f1, b_f1, Wi_b1, Wh_b1, b_b1):
    T = x.shape[1]
    x = np.asarray(x, dtype=np.float32)
    params0 = [_prep_weights(Wi_f0, Wh_f0, b_f0, 2),
               _prep_weights(Wi_b0, Wh_b0, b_b0, 2)]
    params1 = [_prep_weights(Wi_f1, Wh_f1, b_f1, 8),
               _prep_weights(Wi_b1, Wh_b1, b_b1, 8)]
    nc = _get_merged(T)
    in_maps = []
    for c in range(8):
        d, s = c // 4, c % 4
        xs = x[s * BL:(s + 1) * BL]
        xt = xs.transpose(2, 1, 0)
        if d == 1:
            xt = xt[:, ::-1, :]
        xt = np.ascontiguousarray(xt).astype(NP_BF16).reshape(2, 128, T * 8)
        wi0, wh0, bT0 = params0[d]
        wi1, wh1, bT1 = params1[d]
        in_maps.append({"xT": xt, "wh0": wh0, "wi0": wi0, "bT0": bT0,
                        "wh1": wh1, "wi1": wi1, "bT1": bT1})
    res = _run(nc, in_maps)

    out = np.empty((B_GLOBAL, T, 2 * H), np.float32)
    for c in range(8):
        d, s = c // 4, c % 4
        a = res[c]["hout"].reshape(128, T, 4, 8)
        if d == 1:
            a = a[:, ::-1]
        blk = a.transpose(3, 1, 2, 0).reshape(BL, T, H)
        out[s * BL:(s + 1) * BL, :, d * H:(d + 1) * H] = blk
    return out


def _prep_weights(Wi, Wh, b, dc_n):
    s = WH_SCALE
    wi = np.ascontiguousarray(
        (np.asarray(Wi, np.float32) * s)[:, _PERM]
    ).astype(NP_BF16).reshape(dc_n, 128, G)
    wh = np.ascontiguousarray(
        (np.asarray(Wh, np.float32) * s)[:, _PERM]
    ).astype(NP_E3M4).reshape(N_KC, 128, G)
    bT = np.ascontiguousarray(
        (np.asarray(b, np.float32) * s)[_PERM].reshape(N_MC, 128).T
    ).astype(np.float32)
    return wi, wh, bT


def _run(nc, in_maps):
    import time
    t0 = time.time()
    res = bass_utils.run_bass_kernel_spmd(
        nc, in_maps, core_ids=list(range(8)), trace=TRACE)
    LAST_WALL.append(time.time() - t0)
    if TRACE:
        LAST_RESULTS.append(res)
    return res.results


def kernel(x, Wi_f0, Wh_f0, b_f0, Wi_b0, Wh_b0, b_b0,
           Wi_f1, Wh_f1, b_f1, Wi_b1, Wh_b1, b_b1):
    T = x.shape[1]
    x = np.asarray(x, dtype=np.float32)

    # ---------------- layer 0 ----------------
    params0 = [_prep_weights(Wi_f0, Wh_f0, b_f0, 2),
               _prep_weights(Wi_b0, Wh_b0, b_b0, 2)]
    nc0 = _get_program(0, T)
    in_maps = []
    for c in range(8):
        d, s = c // 4, c % 4
        xs = x[s * BL:(s + 1) * BL]            # [8, T, 256]
        xt = xs.transpose(2, 1, 0)             # [256, T, 8] = (d, t, b)
        if d == 1:
            xt = xt[:, ::-1, :]
        xt = np.ascontiguousarray(xt).astype(NP_BF16).reshape(2, 128, T * 8)
        wi, wh, bT = params0[d]
        in_maps.append({"xT": xt, "wh": wh, "wi": wi, "bT": bT})
    res0 = _run(nc0, in_maps)

    # assemble layer-1 inputs: hidden0^T = [h_fwd ; h_bwd] along feature dim
    def to_kptb(a, flip):                      # [128, T*32] -> [4, 128, T, 8]
        a = a.reshape(128, T, 4, 8)
        if flip:
            a = a[:, ::-1]
        return a.transpose(2, 0, 1, 3)
    hidden0T = []
    for s in range(4):
        hf = to_kptb(res0[s]["hout"], False)
        hb = to_kptb(res0[4 + s]["hout"], True)
        hidden0T.append(np.concatenate([hf, hb], axis=0))   # [8, 128, T, 8]

    # ---------------- layer 1 ----------------
    params1 = [_prep_weights(Wi_f1, Wh_f1, b_f1, 8),
               _prep_weights(Wi_b1, Wh_b1, b_b1, 8)]
    nc1 = _get_program(1, T)
    in_maps = []
    for c in range(8):
        d, s = c // 4, c % 4
        ht = hidden0T[s]
        if d == 1:
            ht = ht[:, :, ::-1, :]
        xt = np.ascontiguousarray(ht).reshape(8, 128, T * 8)
        wi, wh, bT = params1[d]
        in_maps.append({"xT": xt, "wh": wh, "wi": wi, "bT": bT})
    res1 = _run(nc1, in_maps)

    # ---------------- final assembly ----------------
    out = np.empty((B_GLOBAL, T, 2 * H), np.float32)
    for c in range(8):
        d, s = c // 4, c % 4
        a = res1[c]["hout"].reshape(128, T, 4, 8)
        if d == 1:
            a = a[:, ::-1]
        blk = a.transpose(3, 1, 2, 0).reshape(BL, T, H)
        out[s * BL:(s + 1) * BL, :, d * H:(d + 1) * H] = blk
    return out

